# revision 16
# baseline (speedup 1.0000x reference)
"""Self-contained Trainium2 Bass kernel: 16-head attention with RoPE (B=2, S=2048, D=2048).

Sharding: 8 cores = 2 (batch) x 4 (head groups of 4 heads / 512 cols).
Per core: QKV projections for its head group -> RoPE -> causal attention ->
AllGather of attention outputs (X^T) within the 4-core batch group ->
column-sharded output projection. Host assembles by concatenation only.

Dataflow is fully "transposed" so no on-chip transposes are needed:
  hiddenT [d, s] (host-pretransposed, bf16)
  QT/KT   [dh, s] per head  (projection emits head-dim-major directly)
  S^T     [k, q] scores     (lhsT = KT tile, rhs = QT)
  P^T     [k, q] = exp(S^T + mask^T)   (no max subtraction; scores are O(1))
  rowsums via ones-vector matmul (partition-axis reduce on PE)
  O^T     [dh, q] = V^T @ P^T          (lhsT = V natural [s, dh])
  X^T     AllGather on first axis -> [D, S]
  out     [s, oc] (lhsT = X^T block, rhs = WoT)
RoPE de-interleave is folded into a host-side row permutation of Wq/Wk.
1/sqrt(DH) is folded into the Q rope tables.
"""

import math
from contextlib import ExitStack

import numpy as np
import ml_dtypes

B, S, D, H, DH = 2, 2048, 2048, 16, 128
NCORES = 8
GPC = 4            # cores per tensor-parallel group
HPC = H // GPC     # heads per core (4)
CW = HPC * DH      # 512 columns per core
NEG = -1e9
BF = ml_dtypes.bfloat16
QCH = 512          # q-chunk (moving free dim)
NQC = S // QCH     # 4
NDT = D // 128     # 16 d-tiles
NST = S // 128     # 16 s-tiles

REPLICA_GROUPS = [[0, 1, 2, 3], [4, 5, 6, 7]]

_built = {}


def _build(causal: bool):
    import concourse.bass as bass
    import concourse.tile as tile
    from concourse import bacc, mybir

    f32, bf16 = mybir.dt.float32, mybir.dt.bfloat16
    EXP = mybir.ActivationFunctionType.Exp
    IDN = mybir.ActivationFunctionType.Identity

    nc = bacc.Bacc("TRN2", target_bir_lowering=False, debug=False,
                   num_devices=NCORES)

    hT_d = nc.dram_tensor("hiddenT", [D, S], bf16, kind="ExternalInput")
    wq_d = nc.dram_tensor("wqT", [D, CW], bf16, kind="ExternalInput")
    wk_d = nc.dram_tensor("wkT", [D, CW], bf16, kind="ExternalInput")
    wv_d = nc.dram_tensor("wvT", [D, CW], bf16, kind="ExternalInput")
    wo_d = nc.dram_tensor("woT", [D, CW], bf16, kind="ExternalInput")
    cq_d = nc.dram_tensor("cq", [128, S], bf16, kind="ExternalInput")
    sq_d = nc.dram_tensor("sq", [128, S], bf16, kind="ExternalInput")
    ck_d = nc.dram_tensor("ck", [128, S], bf16, kind="ExternalInput")
    sk_d = nc.dram_tensor("sk", [128, S], bf16, kind="ExternalInput")
    bq_d = nc.dram_tensor("bqp", [128, HPC], f32, kind="ExternalInput")
    bk_d = nc.dram_tensor("bkp", [128, HPC], f32, kind="ExternalInput")
    bv_d = nc.dram_tensor("bv2", [1, CW], f32, kind="ExternalInput")
    bo_d = nc.dram_tensor("bo2", [1, CW], f32, kind="ExternalInput")
    if causal:
        dm_d = nc.dram_tensor("dmask", [GPC, 128, QCH], bf16, kind="ExternalInput")
    else:
        mT_d = nc.dram_tensor("maskT", [S, S], bf16, kind="ExternalInput")
    out_d = nc.dram_tensor("out", [S, CW], f32, kind="ExternalOutput")

    with tile.TileContext(nc) as tc, ExitStack() as ctx:
        big = ctx.enter_context(tc.tile_pool(name="big", bufs=NDT))
        wp = ctx.enter_context(tc.tile_pool(name="wp", bufs=NDT))
        qkp = ctx.enter_context(tc.tile_pool(name="qkp", bufs=2 * HPC))
        stg = ctx.enter_context(tc.tile_pool(name="stg", bufs=2))
        vp = ctx.enter_context(tc.tile_pool(name="vp", bufs=NST))
        cst = ctx.enter_context(tc.tile_pool(name="cst", bufs=1))
        ptp = ctx.enter_context(tc.tile_pool(name="ptp", bufs=3))
        rp = ctx.enter_context(tc.tile_pool(name="rp", bufs=2))
        op = ctx.enter_context(tc.tile_pool(name="op", bufs=3))
        ps_mm = ctx.enter_context(tc.tile_pool(name="ps_mm", bufs=3, space="PSUM"))
        ps_s = ctx.enter_context(tc.tile_pool(name="ps_s", bufs=2, space="PSUM"))
        ps_pv = ctx.enter_context(tc.tile_pool(name="ps_pv", bufs=2, space="PSUM"))
        ps_sum = ctx.enter_context(tc.tile_pool(name="ps_sum", bufs=1, space="PSUM"))
        dram = ctx.enter_context(tc.tile_pool(name="dram", bufs=1, space="DRAM"))

        # ---- constants ----
        cq_sb = cst.tile([128, S], bf16, tag="cq", name="cq_sb")
        sq_sb = cst.tile([128, S], bf16, tag="sq", name="sq_sb")
        ck_sb = cst.tile([128, S], bf16, tag="ck", name="ck_sb")
        sk_sb = cst.tile([128, S], bf16, tag="sk", name="sk_sb")
        nc.sync.dma_start(cq_sb[:], cq_d[:])
        nc.sync.dma_start(sq_sb[:], sq_d[:])
        nc.sync.dma_start(ck_sb[:], ck_d[:])
        nc.sync.dma_start(sk_sb[:], sk_d[:])
        bq_sb = cst.tile([128, HPC], f32, tag="bq", name="bq_sb")
        bk_sb = cst.tile([128, HPC], f32, tag="bk", name="bk_sb")
        bv_sb = cst.tile([1, CW], f32, tag="bv", name="bv_sb")
        bo_sb = cst.tile([1, CW], f32, tag="bo", name="bo_sb")
        nc.sync.dma_start(bq_sb[:], bq_d[:])
        nc.sync.dma_start(bk_sb[:], bk_d[:])
        nc.sync.dma_start(bv_sb[:], bv_d[:])
        nc.sync.dma_start(bo_sb[:], bo_d[:])
        bvb_sb = cst.tile([128, CW], f32, tag="bvb", name="bvb_sb")
        bob_sb = cst.tile([128, CW], f32, tag="bob", name="bob_sb")
        nc.gpsimd.partition_broadcast(bvb_sb[:], bv_sb[0:1, :])
        nc.gpsimd.partition_broadcast(bob_sb[:], bo_sb[0:1, :])
        ones_sb = cst.tile([128, 1], bf16, tag="ones", name="ones_sb")
        nc.vector.memset(ones_sb[:], 1.0)
        if causal:
            dm_sb = []
            for p in range(GPC):
                t = cst.tile([128, QCH], bf16, tag=f"dm{p}", name=f"dm{p}_sb")
                nc.sync.dma_start(t[:], dm_d[p])
                dm_sb.append(t)

        # ---- hidden^T resident ----
        hT = []
        for dt in range(NDT):
            t = big.tile([128, S], bf16, tag="big", name=f"hT{dt}")
            nc.sync.dma_start(t[:], hT_d[dt * 128:(dt + 1) * 128, :])
            hT.append(t)

        # ---- V projection (natural [s, vc]) ----
        wv_sb = []
        for dt in range(NDT):
            t = wp.tile([128, CW], bf16, tag="w", name=f"wv{dt}")
            nc.sync.dma_start(t[:], wv_d[dt * 128:(dt + 1) * 128, :])
            wv_sb.append(t)
        v_sb = []
        for st in range(NST):
            ps = ps_mm.tile([128, CW], f32, tag="mm", name=f"psv{st}")
            for dt in range(NDT):
                nc.tensor.matmul(ps[:], hT[dt][:, st * 128:(st + 1) * 128],
                                 wv_sb[dt][:], start=(dt == 0), stop=(dt == NDT - 1))
            vt = vp.tile([128, CW], bf16, tag="v", name=f"v{st}")
            nc.vector.tensor_add(vt[:], ps[:], bvb_sb[:])
            v_sb.append(vt)

        # ---- Q/K projections (head-dim-major [qc, s]) + RoPE ----
        def project_qk(w_dram, b_sb, c_sb, s_sb, prefix):
            w_sb = []
            for dt in range(NDT):
                t = wp.tile([128, CW], bf16, tag="w", name=f"{prefix}w{dt}")
                nc.sync.dma_start(t[:], w_dram[dt * 128:(dt + 1) * 128, :])
                w_sb.append(t)
            out_tiles = []
            for m in range(HPC):
                st_t = stg.tile([128, S], bf16, tag="stg", name=f"{prefix}st{m}")
                for chunk in range(NQC):
                    ps = ps_mm.tile([128, QCH], f32, tag="mm",
                                    name=f"{prefix}ps{m}_{chunk}")
                    for dt in range(NDT):
                        nc.tensor.matmul(
                            ps[:], w_sb[dt][:, m * 128:(m + 1) * 128],
                            hT[dt][:, chunk * QCH:(chunk + 1) * QCH],
                            start=(dt == 0), stop=(dt == NDT - 1))
                    nc.scalar.activation(
                        st_t[:, chunk * QCH:(chunk + 1) * QCH], ps[:], IDN,
                        bias=b_sb[:, m:m + 1])
                # RoPE: rows 0:64 = "real" lanes, 64:128 = "imag" lanes.
                # DVE lanes are partition-fixed, so first swap-copy the halves
                # via SBUF->SBUF DMA, then all binary ops are base-aligned.
                rt = qkp.tile([128, S], bf16, tag="qk", name=f"{prefix}r{m}")
                t1 = rp.tile([128, S], bf16, tag="t1", name=f"{prefix}t1_{m}")
                t2 = rp.tile([128, S], bf16, tag="t2", name=f"{prefix}t2_{m}")
                # t1[0:64] = imag half, t1[64:128] = real half (swapped copy)
                nc.sync.dma_start(t1[0:64, :], st_t[64:128, :])
                nc.sync.dma_start(t1[64:128, :], st_t[0:64, :])
                # real: rt[0:64] = a*cos - b*sin
                nc.vector.tensor_mul(rt[0:64, :], st_t[0:64, :], c_sb[0:64, :])
                nc.vector.tensor_mul(t2[0:64, :], t1[0:64, :], s_sb[0:64, :])
                nc.vector.tensor_sub(rt[0:64, :], rt[0:64, :], t2[0:64, :])
                # imag: rt[64:128] = a*sin + b*cos
                nc.vector.tensor_mul(rt[64:128, :], t1[64:128, :],
                                     s_sb[64:128, :])
                nc.vector.tensor_mul(t2[64:128, :], st_t[64:128, :],
                                     c_sb[64:128, :])
                nc.vector.tensor_add(rt[64:128, :], rt[64:128, :],
                                     t2[64:128, :])
                out_tiles.append(rt)
            return out_tiles

        qtr = project_qk(wq_d, bq_sb, cq_sb, sq_sb, "q")
        ktr = project_qk(wk_d, bk_sb, ck_sb, sk_sb, "k")

        # prefetch Wo while attention runs
        wo_sb = []
        for dt in range(NDT):
            t = wp.tile([128, CW], bf16, tag="w", name=f"wo{dt}")
            nc.sync.dma_start(t[:], wo_d[dt * 128:(dt + 1) * 128, :])
            wo_sb.append(t)

        # ---- attention (S^T layout) + AllGather bounce ----
        agin = dram.tile([CW, S], bf16, name="agin")
        agout = dram.tile([D, S], bf16, name="agout")

        for h in range(HPC):
            for qc in range(NQC):
                nk = 4 * qc + 4 if causal else NST
                pv = ps_pv.tile([128, QCH], f32, tag="pv", name=f"pv{h}_{qc}")
                sm = ps_sum.tile([1, QCH], f32, tag="sum", name=f"sm{h}_{qc}")
                for ki in range(nk):
                    ss = ps_s.tile([128, QCH], f32, tag="s", name=f"ss{h}_{qc}_{ki}")
                    nc.tensor.matmul(
                        ss[:], ktr[h][:, ki * 128:(ki + 1) * 128],
                        qtr[h][:, qc * QCH:(qc + 1) * QCH],
                        start=True, stop=True)
                    if causal:
                        p = ki - 4 * qc
                        if p >= 0:
                            nc.vector.tensor_add(ss[:], ss[:], dm_sb[p][:])
                    else:
                        mt = ptp.tile([128, QCH], bf16, tag="mt",
                                      name=f"mt{h}_{qc}_{ki}")
                        nc.sync.dma_start(
                            mt[:], mT_d[ki * 128:(ki + 1) * 128,
                                        qc * QCH:(qc + 1) * QCH])
                        nc.vector.tensor_add(ss[:], ss[:], mt[:])
                    pt = ptp.tile([128, QCH], bf16, tag="pt",
                                  name=f"pt{h}_{qc}_{ki}")
                    nc.scalar.activation(pt[:], ss[:], EXP)
                    nc.tensor.matmul(sm[0:1, :], ones_sb[:], pt[:],
                                     start=(ki == 0), stop=(ki == nk - 1))
                    nc.tensor.matmul(pv[:], v_sb[ki][:, h * 128:(h + 1) * 128],
                                     pt[:], start=(ki == 0), stop=(ki == nk - 1))
                rec = ptp.tile([1, QCH], f32, tag="rec", name=f"rec{h}_{qc}",
                               bufs=2)
                nc.vector.reciprocal(rec[:], sm[0:1, :])
                recb = ptp.tile([128, QCH], f32, tag="recb",
                                name=f"recb{h}_{qc}", bufs=2)
                nc.gpsimd.partition_broadcast(recb[:], rec[0:1, :])
                ot = op.tile([128, QCH], bf16, tag="ot", name=f"ot{h}_{qc}")
                nc.vector.tensor_mul(ot[:], pv[:], recb[:])
                nc.sync.dma_start(
                    agin[h * 128:(h + 1) * 128, qc * QCH:(qc + 1) * QCH], ot[:])

        nc.gpsimd.collective_compute(
            "AllGather", mybir.AluOpType.bypass,
            replica_groups=REPLICA_GROUPS,
            ins=[agin[:].opt()], outs=[agout[:].opt()])

        # ---- output projection ----
        xt = []
        for dt in range(NDT):
            t = big.tile([128, S], bf16, tag="big", name=f"xt{dt}")
            nc.sync.dma_start(t[:], agout[dt * 128:(dt + 1) * 128, :])
            xt.append(t)
        for st in range(NST):
            ps = ps_mm.tile([128, CW], f32, tag="mm", name=f"pso{st}")
            for dt in range(NDT):
                nc.tensor.matmul(ps[:], xt[dt][:, st * 128:(st + 1) * 128],
                                 wo_sb[dt][:], start=(dt == 0), stop=(dt == NDT - 1))
            of = op.tile([128, CW], f32, tag="of", name=f"of{st}", bufs=2)
            nc.vector.tensor_add(of[:], ps[:], bob_sb[:])
            nc.sync.dma_start(out_d[st * 128:(st + 1) * 128, :], of[:])

    nc.compile()
    return nc


def _get_built(causal: bool):
    if causal not in _built:
        _built[causal] = _build(causal)
    return _built[causal]


def _prep_inputs(inputs, causal):
    hs = np.asarray(inputs["hidden_states"], np.float32)
    fc = np.asarray(inputs["freqs_cis"], np.float32)
    Wq = np.asarray(inputs["Wq"], np.float32)
    Wk = np.asarray(inputs["Wk"], np.float32)
    Wv = np.asarray(inputs["Wv"], np.float32)
    Wo = np.asarray(inputs["Wo"], np.float32)
    bq = np.asarray(inputs["bq"], np.float32)
    bk = np.asarray(inputs["bk"], np.float32)
    bv = np.asarray(inputs["bv"], np.float32)
    bo = np.asarray(inputs["bo"], np.float32)

    # de-interleave permutation per 128-row head block: [0,2,..,126, 1,3,..,127]
    perm1 = np.concatenate([np.arange(0, DH, 2), np.arange(1, DH, 2)])
    permC = (np.arange(CW) // DH) * DH  # head base offsets
    perm = permC + perm1[np.arange(CW) % DH]

    scale = 1.0 / math.sqrt(DH)
    cos = np.concatenate([fc[:, :, 0].T, fc[:, :, 0].T])  # [128, S], dup halves
    sin = np.concatenate([fc[:, :, 1].T, fc[:, :, 1].T])
    cq = np.ascontiguousarray(cos * scale).astype(BF)
    sq = np.ascontiguousarray(sin * scale).astype(BF)
    ck = np.ascontiguousarray(cos).astype(BF)
    sk = np.ascontiguousarray(sin).astype(BF)

    if causal:
        dmask = np.stack([
            np.where(np.arange(128)[:, None] + 128 * p > np.arange(QCH)[None, :],
                     np.float32(NEG), np.float32(0.0))
            for p in range(GPC)]).astype(BF)
    else:
        maskT = np.ascontiguousarray(
            np.asarray(inputs["mask"], np.float32)[0, 0].T).astype(BF)

    hTb = [np.ascontiguousarray(hs[b].T).astype(BF) for b in range(B)]

    in_maps = []
    for c in range(NCORES):
        b, hg = divmod(c, GPC)
        sl = slice(CW * hg, CW * (hg + 1))
        wq_s = Wq[sl][perm]
        wk_s = Wk[sl][perm]
        m = {
            "hiddenT": hTb[b],
            "wqT": np.ascontiguousarray(wq_s.T).astype(BF),
            "wkT": np.ascontiguousarray(wk_s.T).astype(BF),
            "wvT": np.ascontiguousarray(Wv[sl].T).astype(BF),
            "woT": np.ascontiguousarray(Wo[sl].T).astype(BF),
            "cq": cq, "sq": sq, "ck": ck, "sk": sk,
            "bqp": np.ascontiguousarray(
                bq[sl][perm].reshape(HPC, 128).T).astype(np.float32),
            "bkp": np.ascontiguousarray(
                bk[sl][perm].reshape(HPC, 128).T).astype(np.float32),
            "bv2": bv[sl].reshape(1, CW).astype(np.float32),
            "bo2": bo[sl].reshape(1, CW).astype(np.float32),
        }
        if causal:
            m["dmask"] = dmask
        else:
            m["maskT"] = maskT
        in_maps.append(m)
    return in_maps


def _is_causal(mask):
    mask = np.asarray(mask, np.float32)
    if mask.shape != (1, 1, S, S):
        return False
    m = mask[0, 0]
    expect = np.triu(np.full((S, S), np.float32(NEG)), k=1)
    return bool(np.array_equal(m, expect))


def run_on_cores(inputs, trace=False):
    """Compile+run; returns (results, BassKernelResults)."""
    from concourse.bass_utils import run_bass_kernel_spmd
    causal = _is_causal(inputs["mask"])
    nc = _get_built(causal)
    in_maps = _prep_inputs(inputs, causal)
    r = run_bass_kernel_spmd(nc, in_maps, list(range(NCORES)), trace=trace)
    return r


def kernel(**inputs) -> np.ndarray:
    r = run_on_cores(inputs)
    out = np.empty((B, S, D), np.float32)
    for c in range(NCORES):
        b, hg = divmod(c, GPC)
        out[b, :, CW * hg:CW * (hg + 1)] = r.results[c]["out"]
    return out


# revision 20
# speedup vs baseline: 1.2054x; 1.2054x over previous
"""Self-contained Trainium2 Bass kernel: 16-head attention with RoPE (B=2, S=2048, D=2048).

Sharding: 8 cores = 2 (batch) x 4 (head groups of 4 heads / 512 cols).
Per core: QKV projections for its head group -> RoPE -> causal attention ->
AllGather of attention outputs (X^T) within the 4-core batch group ->
column-sharded output projection. Host assembles by concatenation only.

Dataflow is fully "transposed" so no on-chip transposes are needed:
  hiddenT [d, s] (host-pretransposed, bf16)
  QT/KT   [dh, s] per head  (projection emits head-dim-major directly)
  S^T     [k, q] scores     (lhsT = KT tile, rhs = QT)
  P^T     [k, q] = exp(S^T + mask^T)   (no max subtraction; scores are O(1))
  colsums via all-ones [128,128] matmul -> sums arrive partition-broadcast
  O^T     [dh, q] = V^T @ P^T          (lhsT = V natural [s, dh])
  X^T     AllGather on first axis, chunked along q and pipelined vs attention
  out     [s, oc] (lhsT = X^T block, rhs = WoT)
RoPE de-interleave is folded into a host-side row permutation of Wq/Wk.
1/sqrt(DH) is folded into the Q rope tables.
"""

import math
from contextlib import ExitStack

import numpy as np
import ml_dtypes

B, S, D, H, DH = 2, 2048, 2048, 16, 128
NCORES = 8
GPC = 4            # cores per tensor-parallel group
HPC = H // GPC     # heads per core (4)
CW = HPC * DH      # 512 columns per core
NEG = -1e9
BF = ml_dtypes.bfloat16
QCH = 512          # q-chunk (moving free dim)
NQC = S // QCH     # 4
NDT = D // 128     # 16 d-tiles
NST = S // 128     # 16 s-tiles

REPLICA_GROUPS = [[0, 1, 2, 3], [4, 5, 6, 7]]

_built = {}


def _build(causal: bool, use_bias: bool):
    import concourse.bass as bass
    import concourse.tile as tile
    from concourse import bacc, mybir

    f32, bf16 = mybir.dt.float32, mybir.dt.bfloat16
    EXP = mybir.ActivationFunctionType.Exp
    IDN = mybir.ActivationFunctionType.Identity
    RCP = mybir.ActivationFunctionType.Reciprocal

    nc = bacc.Bacc("TRN2", target_bir_lowering=False, debug=False,
                   num_devices=NCORES)

    hT_d = nc.dram_tensor("hiddenT", [D, S], bf16, kind="ExternalInput")
    wq_d = nc.dram_tensor("wqT", [D, CW], bf16, kind="ExternalInput")
    wk_d = nc.dram_tensor("wkT", [D, CW], bf16, kind="ExternalInput")
    wv_d = nc.dram_tensor("wvT", [D, CW], bf16, kind="ExternalInput")
    wo_d = nc.dram_tensor("woT", [D, CW], bf16, kind="ExternalInput")
    cq_d = nc.dram_tensor("cq", [128, S], bf16, kind="ExternalInput")
    sq_d = nc.dram_tensor("sq", [128, S], bf16, kind="ExternalInput")
    ck_d = nc.dram_tensor("ck", [128, S], bf16, kind="ExternalInput")
    sk_d = nc.dram_tensor("sk", [128, S], bf16, kind="ExternalInput")
    if use_bias:
        bq_d = nc.dram_tensor("bqp", [128, HPC], f32, kind="ExternalInput")
        bk_d = nc.dram_tensor("bkp", [128, HPC], f32, kind="ExternalInput")
        bv_d = nc.dram_tensor("bv2", [1, CW], f32, kind="ExternalInput")
        bo_d = nc.dram_tensor("bo2", [1, CW], f32, kind="ExternalInput")
    if causal:
        dm_d = nc.dram_tensor("dmask", [128, 128], bf16, kind="ExternalInput")
    else:
        mT_d = nc.dram_tensor("maskT", [S, S], bf16, kind="ExternalInput")
    out_d = nc.dram_tensor("out", [S, CW], f32, kind="ExternalOutput")

    with tile.TileContext(nc) as tc, ExitStack() as ctx:
        big = ctx.enter_context(tc.tile_pool(name="big", bufs=NDT))
        wp = ctx.enter_context(tc.tile_pool(name="wp", bufs=NDT))
        qkp = ctx.enter_context(tc.tile_pool(name="qkp", bufs=2 * HPC))
        stg = ctx.enter_context(tc.tile_pool(name="stg", bufs=2))
        vp = ctx.enter_context(tc.tile_pool(name="vp", bufs=NST))
        cst = ctx.enter_context(tc.tile_pool(name="cst", bufs=1))
        ptp = ctx.enter_context(tc.tile_pool(name="ptp", bufs=3))
        rp = ctx.enter_context(tc.tile_pool(name="rp", bufs=2))
        op = ctx.enter_context(tc.tile_pool(name="op", bufs=3))
        ps_mm = ctx.enter_context(tc.tile_pool(name="ps_mm", bufs=2, space="PSUM"))
        ps_s = ctx.enter_context(tc.tile_pool(name="ps_s", bufs=2, space="PSUM"))
        ps_pv = ctx.enter_context(tc.tile_pool(name="ps_pv", bufs=2, space="PSUM"))
        ps_sum = ctx.enter_context(tc.tile_pool(name="ps_sum", bufs=2, space="PSUM"))
        dram = ctx.enter_context(tc.tile_pool(name="dram", bufs=1, space="DRAM"))

        # ---- constants ----
        cq_sb = cst.tile([128, S], bf16, tag="cq", name="cq_sb")
        sq_sb = cst.tile([128, S], bf16, tag="sq", name="sq_sb")
        ck_sb = cst.tile([128, S], bf16, tag="ck", name="ck_sb")
        sk_sb = cst.tile([128, S], bf16, tag="sk", name="sk_sb")
        nc.sync.dma_start(cq_sb[:], cq_d[:])
        nc.sync.dma_start(sq_sb[:], sq_d[:])
        nc.sync.dma_start(ck_sb[:], ck_d[:])
        nc.sync.dma_start(sk_sb[:], sk_d[:])
        if use_bias:
            bq_sb = cst.tile([128, HPC], f32, tag="bq", name="bq_sb")
            bk_sb = cst.tile([128, HPC], f32, tag="bk", name="bk_sb")
            bv_sb = cst.tile([1, CW], f32, tag="bv", name="bv_sb")
            bo_sb = cst.tile([1, CW], f32, tag="bo", name="bo_sb")
            nc.sync.dma_start(bq_sb[:], bq_d[:])
            nc.sync.dma_start(bk_sb[:], bk_d[:])
            nc.sync.dma_start(bv_sb[:], bv_d[:])
            nc.sync.dma_start(bo_sb[:], bo_d[:])
            bvb_sb = cst.tile([128, CW], f32, tag="bvb", name="bvb_sb")
            bob_sb = cst.tile([128, CW], f32, tag="bob", name="bob_sb")
            nc.gpsimd.partition_broadcast(bvb_sb[:], bv_sb[0:1, :])
            nc.gpsimd.partition_broadcast(bob_sb[:], bo_sb[0:1, :])
        ones_sb = cst.tile([128, 128], bf16, tag="ones", name="ones_sb")
        nc.vector.memset(ones_sb[:], 1.0)
        if causal:
            tri_sb = cst.tile([128, 128], bf16, tag="tri", name="tri_sb")
            nc.sync.dma_start(tri_sb[:], dm_d[:])

        # ---- hidden^T + Wv resident (interleaved so dt=0 arrives first) ----
        hT = []
        wv_sb = []
        for dt in range(NDT):
            t = big.tile([128, S], bf16, tag="big", name=f"hT{dt}")
            nc.sync.dma_start(t[:], hT_d[dt * 128:(dt + 1) * 128, :])
            hT.append(t)
            w = wp.tile([128, CW], bf16, tag="w", name=f"wv{dt}")
            nc.sync.dma_start(w[:], wv_d[dt * 128:(dt + 1) * 128, :])
            wv_sb.append(w)

        # ---- V projection (natural [s, vc]) ----
        v_sb = []
        for st in range(NST):
            ps = ps_mm.tile([128, CW], f32, tag="mm", name=f"psv{st}")
            for dt in range(NDT):
                nc.tensor.matmul(ps[:], hT[dt][:, st * 128:(st + 1) * 128],
                                 wv_sb[dt][:], start=(dt == 0), stop=(dt == NDT - 1))
            vt = vp.tile([128, CW], bf16, tag="v", name=f"v{st}")
            if use_bias:
                nc.vector.tensor_add(vt[:], ps[:], bvb_sb[:])
            else:
                nc.scalar.activation(vt[:], ps[:], IDN)
            v_sb.append(vt)

        # ---- Q/K projections (head-dim-major [qc, s]) + RoPE ----
        def project_qk(w_dram, b_sb, c_sb, s_sb, prefix):
            w_sb = []
            for dt in range(NDT):
                t = wp.tile([128, CW], bf16, tag="w", name=f"{prefix}w{dt}")
                nc.sync.dma_start(t[:], w_dram[dt * 128:(dt + 1) * 128, :])
                w_sb.append(t)
            out_tiles = []
            for m in range(HPC):
                st_t = stg.tile([128, S], bf16, tag="stg", name=f"{prefix}st{m}")
                for chunk in range(NQC):
                    ps = ps_mm.tile([128, QCH], f32, tag="mm",
                                    name=f"{prefix}ps{m}_{chunk}")
                    for dt in range(NDT):
                        nc.tensor.matmul(
                            ps[:], w_sb[dt][:, m * 128:(m + 1) * 128],
                            hT[dt][:, chunk * QCH:(chunk + 1) * QCH],
                            start=(dt == 0), stop=(dt == NDT - 1))
                    if use_bias:
                        nc.scalar.activation(
                            st_t[:, chunk * QCH:(chunk + 1) * QCH], ps[:], IDN,
                            bias=b_sb[:, m:m + 1])
                    else:
                        nc.scalar.activation(
                            st_t[:, chunk * QCH:(chunk + 1) * QCH], ps[:], IDN)
                # RoPE: rows 0:64 = "real" lanes, 64:128 = "imag" lanes.
                # DVE lanes are partition-fixed, so first swap-copy the halves
                # via SBUF->SBUF DMA, then all binary ops are base-aligned.
                rt = qkp.tile([128, S], bf16, tag="qk", name=f"{prefix}r{m}")
                t1 = rp.tile([128, S], bf16, tag="t1", name=f"{prefix}t1_{m}")
                t2 = rp.tile([128, S], bf16, tag="t2", name=f"{prefix}t2_{m}")
                # t1[0:64] = imag half, t1[64:128] = real half (swapped copy)
                nc.sync.dma_start(t1[0:64, :], st_t[64:128, :])
                nc.sync.dma_start(t1[64:128, :], st_t[0:64, :])
                # real: rt[0:64] = a*cos - b*sin
                nc.vector.tensor_mul(rt[0:64, :], st_t[0:64, :], c_sb[0:64, :])
                nc.vector.tensor_mul(t2[0:64, :], t1[0:64, :], s_sb[0:64, :])
                nc.vector.tensor_sub(rt[0:64, :], rt[0:64, :], t2[0:64, :])
                # imag: rt[64:128] = a*sin + b*cos
                nc.vector.tensor_mul(rt[64:128, :], t1[64:128, :],
                                     s_sb[64:128, :])
                nc.vector.tensor_mul(t2[64:128, :], st_t[64:128, :],
                                     c_sb[64:128, :])
                nc.vector.tensor_add(rt[64:128, :], rt[64:128, :],
                                     t2[64:128, :])
                out_tiles.append(rt)
            return out_tiles

        qtr = project_qk(wq_d, bq_sb if use_bias else None, cq_sb, sq_sb, "q")
        ktr = project_qk(wk_d, bk_sb if use_bias else None, ck_sb, sk_sb, "k")

        # prefetch Wo while attention runs
        wo_sb = []
        for dt in range(NDT):
            t = wp.tile([128, CW], bf16, tag="w", name=f"wo{dt}")
            nc.sync.dma_start(t[:], wo_d[dt * 128:(dt + 1) * 128, :])
            wo_sb.append(t)

        # ---- attention (S^T layout), q-chunk outer; AG + outproj pipelined ----
        for qc in range(NQC):
            agin = dram.tile([CW, QCH], bf16, tag=f"agin{qc}",
                             name=f"agin{qc}")
            agout = dram.tile([D, QCH], bf16, tag=f"agout{qc}",
                              name=f"agout{qc}")
            for h in range(HPC):
                nk = 4 * qc + 4 if causal else NST
                pv = ps_pv.tile([128, QCH], f32, tag="pv", name=f"pv{h}_{qc}")
                sm = ps_sum.tile([128, QCH], f32, tag="sum", name=f"sm{h}_{qc}")
                for ki in range(nk):
                    p = ki - 4 * qc if causal else -1
                    c0 = max(0, 128 * p)
                    ss = ps_s.tile([128, QCH], f32, tag="s",
                                   name=f"ss{h}_{qc}_{ki}")
                    nc.tensor.matmul(
                        ss[:, c0:], ktr[h][:, ki * 128:(ki + 1) * 128],
                        qtr[h][:, qc * QCH + c0:(qc + 1) * QCH],
                        start=True, stop=True)
                    if causal:
                        if p >= 0:
                            nc.vector.tensor_add(ss[:, c0:c0 + 128],
                                                 ss[:, c0:c0 + 128], tri_sb[:])
                    else:
                        mt = ptp.tile([128, QCH], bf16, tag="mt",
                                      name=f"mt{h}_{qc}_{ki}")
                        nc.sync.dma_start(
                            mt[:], mT_d[ki * 128:(ki + 1) * 128,
                                        qc * QCH:(qc + 1) * QCH])
                        nc.vector.tensor_add(ss[:], ss[:], mt[:])
                    pt = ptp.tile([128, QCH], bf16, tag="pt",
                                  name=f"pt{h}_{qc}_{ki}")
                    if c0 > 0:
                        nc.vector.memset(pt[:, 0:c0], 0.0)
                    nc.scalar.activation(pt[:, c0:], ss[:, c0:], EXP)
                    nc.tensor.matmul(sm[:], ones_sb[:], pt[:],
                                     start=(ki == 0), stop=(ki == nk - 1))
                    nc.tensor.matmul(pv[:], v_sb[ki][:, h * 128:(h + 1) * 128],
                                     pt[:], start=(ki == 0), stop=(ki == nk - 1))
                recb = ptp.tile([128, QCH], f32, tag="recb",
                                name=f"recb{h}_{qc}", bufs=2)
                nc.vector.reciprocal(recb[:], sm[:])
                ot = op.tile([128, QCH], bf16, tag="ot", name=f"ot{h}_{qc}")
                nc.vector.tensor_mul(ot[:], pv[:], recb[:])
                nc.sync.dma_start(agin[h * 128:(h + 1) * 128, :], ot[:])

            nc.gpsimd.collective_compute(
                "AllGather", mybir.AluOpType.bypass,
                replica_groups=REPLICA_GROUPS,
                ins=[agin[:].opt()], outs=[agout[:].opt()])

            # ---- output projection for this q-chunk ----
            xt = []
            for dt in range(NDT):
                t = big.tile([128, QCH], bf16, tag="big", name=f"xt{qc}_{dt}")
                nc.sync.dma_start(t[:], agout[dt * 128:(dt + 1) * 128, :])
                xt.append(t)
            for st4 in range(QCH // 128):
                ps = ps_mm.tile([128, CW], f32, tag="mm", name=f"pso{qc}_{st4}")
                for dt in range(NDT):
                    nc.tensor.matmul(
                        ps[:], xt[dt][:, st4 * 128:(st4 + 1) * 128],
                        wo_sb[dt][:], start=(dt == 0), stop=(dt == NDT - 1))
                row = qc * QCH + st4 * 128
                of = op.tile([128, CW], f32, tag="of", name=f"of{qc}_{st4}",
                             bufs=2)
                if use_bias:
                    nc.vector.tensor_add(of[:], ps[:], bob_sb[:])
                else:
                    nc.scalar.activation(of[:], ps[:], IDN)
                nc.sync.dma_start(out_d[row:row + 128, :], of[:])

    nc.compile()
    return nc


def _get_built(causal: bool, use_bias: bool):
    key = (causal, use_bias)
    if key not in _built:
        _built[key] = _build(causal, use_bias)
    return _built[key]


def _prep_inputs(inputs, causal, use_bias):
    hs = np.asarray(inputs["hidden_states"], np.float32)
    fc = np.asarray(inputs["freqs_cis"], np.float32)
    Wq = np.asarray(inputs["Wq"], np.float32)
    Wk = np.asarray(inputs["Wk"], np.float32)
    Wv = np.asarray(inputs["Wv"], np.float32)
    Wo = np.asarray(inputs["Wo"], np.float32)
    bq = np.asarray(inputs["bq"], np.float32)
    bk = np.asarray(inputs["bk"], np.float32)
    bv = np.asarray(inputs["bv"], np.float32)
    bo = np.asarray(inputs["bo"], np.float32)

    # de-interleave permutation per 128-row head block: [0,2,..,126, 1,3,..,127]
    perm1 = np.concatenate([np.arange(0, DH, 2), np.arange(1, DH, 2)])
    permC = (np.arange(CW) // DH) * DH  # head base offsets
    perm = permC + perm1[np.arange(CW) % DH]

    scale = 1.0 / math.sqrt(DH)
    cos = np.concatenate([fc[:, :, 0].T, fc[:, :, 0].T])  # [128, S], dup halves
    sin = np.concatenate([fc[:, :, 1].T, fc[:, :, 1].T])
    cq = np.ascontiguousarray(cos * scale).astype(BF)
    sq = np.ascontiguousarray(sin * scale).astype(BF)
    ck = np.ascontiguousarray(cos).astype(BF)
    sk = np.ascontiguousarray(sin).astype(BF)

    if causal:
        tri = np.where(np.arange(128)[:, None] > np.arange(128)[None, :],
                       np.float32(NEG), np.float32(0.0)).astype(BF)
    else:
        maskT = np.ascontiguousarray(
            np.asarray(inputs["mask"], np.float32)[0, 0].T).astype(BF)

    hTb = [np.ascontiguousarray(hs[b].T).astype(BF) for b in range(B)]

    in_maps = []
    for c in range(NCORES):
        b, hg = divmod(c, GPC)
        sl = slice(CW * hg, CW * (hg + 1))
        wq_s = Wq[sl][perm]
        wk_s = Wk[sl][perm]
        m = {
            "hiddenT": hTb[b],
            "wqT": np.ascontiguousarray(wq_s.T).astype(BF),
            "wkT": np.ascontiguousarray(wk_s.T).astype(BF),
            "wvT": np.ascontiguousarray(Wv[sl].T).astype(BF),
            "woT": np.ascontiguousarray(Wo[sl].T).astype(BF),
            "cq": cq, "sq": sq, "ck": ck, "sk": sk,
        }
        if use_bias:
            m["bqp"] = np.ascontiguousarray(
                bq[sl][perm].reshape(HPC, 128).T).astype(np.float32)
            m["bkp"] = np.ascontiguousarray(
                bk[sl][perm].reshape(HPC, 128).T).astype(np.float32)
            m["bv2"] = bv[sl].reshape(1, CW).astype(np.float32)
            m["bo2"] = bo[sl].reshape(1, CW).astype(np.float32)
        if causal:
            m["dmask"] = tri
        else:
            m["maskT"] = maskT
        in_maps.append(m)
    return in_maps


def _is_causal(mask):
    mask = np.asarray(mask, np.float32)
    if mask.shape != (1, 1, S, S):
        return False
    m = mask[0, 0]
    expect = np.triu(np.full((S, S), np.float32(NEG)), k=1)
    return bool(np.array_equal(m, expect))


def run_on_cores(inputs, trace=False):
    """Compile+run; returns BassKernelResults."""
    from concourse.bass_utils import run_bass_kernel_spmd
    causal = _is_causal(inputs["mask"])
    use_bias = not all(
        not np.any(np.asarray(inputs[k])) for k in ("bq", "bk", "bv", "bo"))
    nc = _get_built(causal, use_bias)
    in_maps = _prep_inputs(inputs, causal, use_bias)
    r = run_bass_kernel_spmd(nc, in_maps, list(range(NCORES)), trace=trace)
    return r


def kernel(**inputs) -> np.ndarray:
    r = run_on_cores(inputs)
    out = np.empty((B, S, D), np.float32)
    for c in range(NCORES):
        b, hg = divmod(c, GPC)
        out[b, :, CW * hg:CW * (hg + 1)] = r.results[c]["out"]
    return out


# revision 23
# speedup vs baseline: 1.3636x; 1.1312x over previous
"""Self-contained Trainium2 Bass kernel: 16-head attention with RoPE (B=2, S=2048, D=2048).

Sharding: 8 cores = 2 (batch) x 4 (head groups of 4 heads / 512 cols).
Per core: QKV projections for its head group -> RoPE -> causal attention ->
AllGather of attention outputs (X^T) within the 4-core batch group ->
column-sharded output projection. Host assembles by concatenation only.

Dataflow is fully "transposed" so no on-chip transposes are needed:
  hiddenT [d, s] (host-pretransposed, bf16)
  QT/KT   [dh, s] per head  (projection emits head-dim-major directly)
  S^T     [k, q] scores     (lhsT = KT tile, rhs = QT)
  P^T     [k, q] = exp(S^T + mask^T)   (no max subtraction; scores are O(1))
  colsums via all-ones [128,128] matmul -> sums arrive partition-broadcast
  O^T     [dh, q] = V^T @ P^T          (lhsT = V natural [s, dh])
  X^T     AllGather on first axis, chunked along q and pipelined vs attention
  out     [s, oc] (lhsT = X^T block, rhs = WoT)
RoPE de-interleave is folded into a host-side row permutation of Wq/Wk.
1/sqrt(DH) is folded into the Q rope tables.
"""

import math
from contextlib import ExitStack

import numpy as np
import ml_dtypes

B, S, D, H, DH = 2, 2048, 2048, 16, 128
NCORES = 8
GPC = 4            # cores per tensor-parallel group
HPC = H // GPC     # heads per core (4)
CW = HPC * DH      # 512 columns per core
NEG = -1e9
BF = ml_dtypes.bfloat16
QCH = 512          # q-chunk (moving free dim)
NQC = S // QCH     # 4
NDT = D // 128     # 16 d-tiles
NST = S // 128     # 16 s-tiles

REPLICA_GROUPS = [[0, 1, 2, 3], [4, 5, 6, 7]]

_built = {}


def _build(causal: bool, use_bias: bool):
    import concourse.bass as bass
    import concourse.tile as tile
    from concourse import bacc, mybir

    f32, bf16 = mybir.dt.float32, mybir.dt.bfloat16
    EXP = mybir.ActivationFunctionType.Exp
    IDN = mybir.ActivationFunctionType.Identity
    RCP = mybir.ActivationFunctionType.Reciprocal

    nc = bacc.Bacc("TRN2", target_bir_lowering=False, debug=False,
                   num_devices=NCORES)

    hT_d = nc.dram_tensor("hiddenT", [D, S], bf16, kind="ExternalInput")
    wq_d = nc.dram_tensor("wqT", [D, CW], bf16, kind="ExternalInput")
    wk_d = nc.dram_tensor("wkT", [D, CW], bf16, kind="ExternalInput")
    wv_d = nc.dram_tensor("wvT", [D, CW], bf16, kind="ExternalInput")
    wo_d = nc.dram_tensor("woT", [D, CW], bf16, kind="ExternalInput")
    cq_d = nc.dram_tensor("cq", [128, S], bf16, kind="ExternalInput")
    sq_d = nc.dram_tensor("sq", [128, S], bf16, kind="ExternalInput")
    ck_d = nc.dram_tensor("ck", [128, S], bf16, kind="ExternalInput")
    sk_d = nc.dram_tensor("sk", [128, S], bf16, kind="ExternalInput")
    if use_bias:
        bq_d = nc.dram_tensor("bqp", [128, HPC], f32, kind="ExternalInput")
        bk_d = nc.dram_tensor("bkp", [128, HPC], f32, kind="ExternalInput")
        bv_d = nc.dram_tensor("bv2", [1, CW], f32, kind="ExternalInput")
        bo_d = nc.dram_tensor("bo2", [1, CW], f32, kind="ExternalInput")
    if causal:
        dm_d = nc.dram_tensor("dmask", [128, 128], bf16, kind="ExternalInput")
    else:
        mT_d = nc.dram_tensor("maskT", [S, S], bf16, kind="ExternalInput")
    out_d = nc.dram_tensor("out", [S, CW], f32, kind="ExternalOutput")

    with tile.TileContext(nc) as tc, ExitStack() as ctx:
        big = ctx.enter_context(tc.tile_pool(name="big", bufs=NDT))
        wp = ctx.enter_context(tc.tile_pool(name="wp", bufs=NDT))
        qkp = ctx.enter_context(tc.tile_pool(name="qkp", bufs=2 * HPC))
        stg = ctx.enter_context(tc.tile_pool(name="stg", bufs=2))
        vp = ctx.enter_context(tc.tile_pool(name="vp", bufs=NST))
        cst = ctx.enter_context(tc.tile_pool(name="cst", bufs=1))
        ptp = ctx.enter_context(tc.tile_pool(name="ptp", bufs=3))
        rp = ctx.enter_context(tc.tile_pool(name="rp", bufs=2))
        op = ctx.enter_context(tc.tile_pool(name="op", bufs=3))
        ps_mm = ctx.enter_context(tc.tile_pool(name="ps_mm", bufs=2, space="PSUM"))
        ps_s = ctx.enter_context(tc.tile_pool(name="ps_s", bufs=2, space="PSUM"))
        ps_pv = ctx.enter_context(tc.tile_pool(name="ps_pv", bufs=2, space="PSUM"))
        ps_sum = ctx.enter_context(tc.tile_pool(name="ps_sum", bufs=2, space="PSUM"))
        dram = ctx.enter_context(tc.tile_pool(name="dram", bufs=1, space="DRAM"))

        # ---- constants ----
        cq_sb = cst.tile([128, S], bf16, tag="cq", name="cq_sb")
        sq_sb = cst.tile([128, S], bf16, tag="sq", name="sq_sb")
        ck_sb = cst.tile([128, S], bf16, tag="ck", name="ck_sb")
        sk_sb = cst.tile([128, S], bf16, tag="sk", name="sk_sb")
        nc.sync.dma_start(cq_sb[:], cq_d[:])
        nc.sync.dma_start(sq_sb[:], sq_d[:])
        nc.sync.dma_start(ck_sb[:], ck_d[:])
        nc.sync.dma_start(sk_sb[:], sk_d[:])
        if use_bias:
            bq_sb = cst.tile([128, HPC], f32, tag="bq", name="bq_sb")
            bk_sb = cst.tile([128, HPC], f32, tag="bk", name="bk_sb")
            bv_sb = cst.tile([1, CW], f32, tag="bv", name="bv_sb")
            bo_sb = cst.tile([1, CW], f32, tag="bo", name="bo_sb")
            nc.sync.dma_start(bq_sb[:], bq_d[:])
            nc.sync.dma_start(bk_sb[:], bk_d[:])
            nc.sync.dma_start(bv_sb[:], bv_d[:])
            nc.sync.dma_start(bo_sb[:], bo_d[:])
            bvb_sb = cst.tile([128, CW], f32, tag="bvb", name="bvb_sb")
            bob_sb = cst.tile([128, CW], f32, tag="bob", name="bob_sb")
            nc.gpsimd.partition_broadcast(bvb_sb[:], bv_sb[0:1, :])
            nc.gpsimd.partition_broadcast(bob_sb[:], bo_sb[0:1, :])
        ones_sb = cst.tile([128, 128], bf16, tag="ones", name="ones_sb")
        nc.vector.memset(ones_sb[:], 1.0)
        if causal:
            tri_sb = cst.tile([128, 128], bf16, tag="tri", name="tri_sb")
            nc.sync.dma_start(tri_sb[:], dm_d[:])

        # ---- hidden^T + Wv resident (interleaved so dt=0 arrives first) ----
        hT = []
        wv_sb = []
        for dt in range(NDT):
            t = big.tile([128, S], bf16, tag="big", name=f"hT{dt}")
            nc.sync.dma_start(t[:], hT_d[dt * 128:(dt + 1) * 128, :])
            hT.append(t)
            w = wp.tile([128, CW], bf16, tag="w", name=f"wv{dt}")
            nc.sync.dma_start(w[:], wv_d[dt * 128:(dt + 1) * 128, :])
            wv_sb.append(w)

        # ---- V projection (natural [s, vc]) ----
        v_sb = []
        for st in range(NST):
            ps = ps_mm.tile([128, CW], f32, tag="mm", name=f"psv{st}")
            for dt in range(NDT):
                nc.tensor.matmul(ps[:], hT[dt][:, st * 128:(st + 1) * 128],
                                 wv_sb[dt][:], start=(dt == 0), stop=(dt == NDT - 1))
            vt = vp.tile([128, CW], bf16, tag="v", name=f"v{st}")
            if use_bias:
                nc.vector.tensor_add(vt[:], ps[:], bvb_sb[:])
            else:
                nc.scalar.activation(vt[:], ps[:], IDN)
            v_sb.append(vt)

        # ---- Q/K projections (head-dim-major [qc, s]) + RoPE ----
        def project_qk(w_dram, b_sb, c_sb, s_sb, prefix):
            w_sb = []
            for dt in range(NDT):
                t = wp.tile([128, CW], bf16, tag="w", name=f"{prefix}w{dt}")
                nc.sync.dma_start(t[:], w_dram[dt * 128:(dt + 1) * 128, :])
                w_sb.append(t)
            out_tiles = []
            for m in range(HPC):
                st_t = stg.tile([128, S], bf16, tag="stg", name=f"{prefix}st{m}")
                for chunk in range(NQC):
                    ps = ps_mm.tile([128, QCH], f32, tag="mm",
                                    name=f"{prefix}ps{m}_{chunk}")
                    for dt in range(NDT):
                        nc.tensor.matmul(
                            ps[:], w_sb[dt][:, m * 128:(m + 1) * 128],
                            hT[dt][:, chunk * QCH:(chunk + 1) * QCH],
                            start=(dt == 0), stop=(dt == NDT - 1))
                    if use_bias:
                        nc.scalar.activation(
                            st_t[:, chunk * QCH:(chunk + 1) * QCH], ps[:], IDN,
                            bias=b_sb[:, m:m + 1])
                    else:
                        nc.scalar.activation(
                            st_t[:, chunk * QCH:(chunk + 1) * QCH], ps[:], IDN)
                # RoPE: rows 0:64 = "real" lanes, 64:128 = "imag" lanes.
                # DVE lanes are partition-fixed, so first swap-copy the halves
                # via SBUF->SBUF DMA, then all binary ops are base-aligned.
                rt = qkp.tile([128, S], bf16, tag="qk", name=f"{prefix}r{m}")
                t1 = rp.tile([128, S], bf16, tag="t1", name=f"{prefix}t1_{m}")
                t2 = rp.tile([128, S], bf16, tag="t2", name=f"{prefix}t2_{m}")
                # t1[0:64] = imag half, t1[64:128] = real half (swapped copy)
                nc.sync.dma_start(t1[0:64, :], st_t[64:128, :])
                nc.sync.dma_start(t1[64:128, :], st_t[0:64, :])
                # real: rt[0:64] = a*cos - b*sin
                nc.vector.tensor_mul(rt[0:64, :], st_t[0:64, :], c_sb[0:64, :])
                nc.vector.tensor_mul(t2[0:64, :], t1[0:64, :], s_sb[0:64, :])
                nc.vector.tensor_sub(rt[0:64, :], rt[0:64, :], t2[0:64, :])
                # imag: rt[64:128] = a*sin + b*cos
                nc.vector.tensor_mul(rt[64:128, :], t1[64:128, :],
                                     s_sb[64:128, :])
                nc.vector.tensor_mul(t2[64:128, :], st_t[64:128, :],
                                     c_sb[64:128, :])
                nc.vector.tensor_add(rt[64:128, :], rt[64:128, :],
                                     t2[64:128, :])
                out_tiles.append(rt)
            return out_tiles

        qtr = project_qk(wq_d, bq_sb if use_bias else None, cq_sb, sq_sb, "q")
        ktr = project_qk(wk_d, bk_sb if use_bias else None, ck_sb, sk_sb, "k")

        # prefetch Wo while attention runs
        wo_sb = []
        for dt in range(NDT):
            t = wp.tile([128, CW], bf16, tag="w", name=f"wo{dt}")
            nc.sync.dma_start(t[:], wo_d[dt * 128:(dt + 1) * 128, :])
            wo_sb.append(t)

        # ---- attention (S^T layout), q-chunk outer; AG + outproj pipelined ----
        # Emission order = per-engine program order. Emit attention(qc+1)
        # BEFORE outproj(qc) so PE never waits on AllGather(qc).
        def attention_chunk(qc):
            agin = dram.tile([CW, QCH], bf16, tag=f"agin{qc}",
                             name=f"agin{qc}")
            agout = dram.tile([D, QCH], bf16, tag=f"agout{qc}",
                              name=f"agout{qc}")
            for h in range(HPC):
                nk = 4 * qc + 4 if causal else NST
                pv = ps_pv.tile([128, QCH], f32, tag="pv", name=f"pv{h}_{qc}")
                sm = ps_sum.tile([128, QCH], f32, tag="sum", name=f"sm{h}_{qc}")
                for ki in range(nk):
                    p = ki - 4 * qc if causal else -1
                    c0 = max(0, 128 * p)
                    ss = ps_s.tile([128, QCH], f32, tag="s",
                                   name=f"ss{h}_{qc}_{ki}")
                    nc.tensor.matmul(
                        ss[:, c0:], ktr[h][:, ki * 128:(ki + 1) * 128],
                        qtr[h][:, qc * QCH + c0:(qc + 1) * QCH],
                        start=True, stop=True)
                    if causal:
                        if p >= 0:
                            nc.vector.tensor_add(ss[:, c0:c0 + 128],
                                                 ss[:, c0:c0 + 128], tri_sb[:])
                    else:
                        mt = ptp.tile([128, QCH], bf16, tag="mt",
                                      name=f"mt{h}_{qc}_{ki}")
                        nc.sync.dma_start(
                            mt[:], mT_d[ki * 128:(ki + 1) * 128,
                                        qc * QCH:(qc + 1) * QCH])
                        nc.vector.tensor_add(ss[:], ss[:], mt[:])
                    pt = ptp.tile([128, QCH], bf16, tag="pt",
                                  name=f"pt{h}_{qc}_{ki}")
                    if c0 > 0:
                        nc.vector.memset(pt[:, 0:c0], 0.0)
                    nc.scalar.activation(pt[:, c0:], ss[:, c0:], EXP)
                    nc.tensor.matmul(sm[:], ones_sb[:], pt[:],
                                     start=(ki == 0), stop=(ki == nk - 1))
                    nc.tensor.matmul(pv[:], v_sb[ki][:, h * 128:(h + 1) * 128],
                                     pt[:], start=(ki == 0), stop=(ki == nk - 1))
                recb = ptp.tile([128, QCH], f32, tag="recb",
                                name=f"recb{h}_{qc}", bufs=2)
                nc.vector.reciprocal_approx_fast(out=recb[:], in_=sm[:])
                ot = op.tile([128, QCH], bf16, tag="ot", name=f"ot{h}_{qc}")
                nc.vector.tensor_mul(ot[:], pv[:], recb[:])
                nc.sync.dma_start(agin[h * 128:(h + 1) * 128, :], ot[:])

            nc.gpsimd.collective_compute(
                "AllGather", mybir.AluOpType.bypass,
                replica_groups=REPLICA_GROUPS,
                ins=[agin[:].opt()], outs=[agout[:].opt()])
            return agout

        def outproj_chunk(qc, agout):
            xt = []
            for dt in range(NDT):
                t = big.tile([128, QCH], bf16, tag="big", name=f"xt{qc}_{dt}")
                nc.sync.dma_start(t[:], agout[dt * 128:(dt + 1) * 128, :])
                xt.append(t)
            for st4 in range(QCH // 128):
                ps = ps_mm.tile([128, CW], f32, tag="mm", name=f"pso{qc}_{st4}")
                for dt in range(NDT):
                    nc.tensor.matmul(
                        ps[:], xt[dt][:, st4 * 128:(st4 + 1) * 128],
                        wo_sb[dt][:], start=(dt == 0), stop=(dt == NDT - 1))
                row = qc * QCH + st4 * 128
                of = op.tile([128, CW], f32, tag="of", name=f"of{qc}_{st4}",
                             bufs=2)
                if use_bias:
                    nc.vector.tensor_add(of[:], ps[:], bob_sb[:])
                else:
                    nc.scalar.activation(of[:], ps[:], IDN)
                nc.sync.dma_start(out_d[row:row + 128, :], of[:])

        agouts = {}
        for qc in range(NQC):
            agouts[qc] = attention_chunk(qc)
            if qc > 0:
                outproj_chunk(qc - 1, agouts[qc - 1])
        outproj_chunk(NQC - 1, agouts[NQC - 1])

    nc.compile()
    return nc


def _get_built(causal: bool, use_bias: bool):
    key = (causal, use_bias)
    if key not in _built:
        _built[key] = _build(causal, use_bias)
    return _built[key]


def _prep_inputs(inputs, causal, use_bias):
    hs = np.asarray(inputs["hidden_states"], np.float32)
    fc = np.asarray(inputs["freqs_cis"], np.float32)
    Wq = np.asarray(inputs["Wq"], np.float32)
    Wk = np.asarray(inputs["Wk"], np.float32)
    Wv = np.asarray(inputs["Wv"], np.float32)
    Wo = np.asarray(inputs["Wo"], np.float32)
    bq = np.asarray(inputs["bq"], np.float32)
    bk = np.asarray(inputs["bk"], np.float32)
    bv = np.asarray(inputs["bv"], np.float32)
    bo = np.asarray(inputs["bo"], np.float32)

    # de-interleave permutation per 128-row head block: [0,2,..,126, 1,3,..,127]
    perm1 = np.concatenate([np.arange(0, DH, 2), np.arange(1, DH, 2)])
    permC = (np.arange(CW) // DH) * DH  # head base offsets
    perm = permC + perm1[np.arange(CW) % DH]

    scale = 1.0 / math.sqrt(DH)
    cos = np.concatenate([fc[:, :, 0].T, fc[:, :, 0].T])  # [128, S], dup halves
    sin = np.concatenate([fc[:, :, 1].T, fc[:, :, 1].T])
    cq = np.ascontiguousarray(cos * scale).astype(BF)
    sq = np.ascontiguousarray(sin * scale).astype(BF)
    ck = np.ascontiguousarray(cos).astype(BF)
    sk = np.ascontiguousarray(sin).astype(BF)

    if causal:
        tri = np.where(np.arange(128)[:, None] > np.arange(128)[None, :],
                       np.float32(NEG), np.float32(0.0)).astype(BF)
    else:
        maskT = np.ascontiguousarray(
            np.asarray(inputs["mask"], np.float32)[0, 0].T).astype(BF)

    hTb = [np.ascontiguousarray(hs[b].T).astype(BF) for b in range(B)]

    in_maps = []
    for c in range(NCORES):
        b, hg = divmod(c, GPC)
        sl = slice(CW * hg, CW * (hg + 1))
        wq_s = Wq[sl][perm]
        wk_s = Wk[sl][perm]
        m = {
            "hiddenT": hTb[b],
            "wqT": np.ascontiguousarray(wq_s.T).astype(BF),
            "wkT": np.ascontiguousarray(wk_s.T).astype(BF),
            "wvT": np.ascontiguousarray(Wv[sl].T).astype(BF),
            "woT": np.ascontiguousarray(Wo[sl].T).astype(BF),
            "cq": cq, "sq": sq, "ck": ck, "sk": sk,
        }
        if use_bias:
            m["bqp"] = np.ascontiguousarray(
                bq[sl][perm].reshape(HPC, 128).T).astype(np.float32)
            m["bkp"] = np.ascontiguousarray(
                bk[sl][perm].reshape(HPC, 128).T).astype(np.float32)
            m["bv2"] = bv[sl].reshape(1, CW).astype(np.float32)
            m["bo2"] = bo[sl].reshape(1, CW).astype(np.float32)
        if causal:
            m["dmask"] = tri
        else:
            m["maskT"] = maskT
        in_maps.append(m)
    return in_maps


def _is_causal(mask):
    mask = np.asarray(mask, np.float32)
    if mask.shape != (1, 1, S, S):
        return False
    m = mask[0, 0]
    expect = np.triu(np.full((S, S), np.float32(NEG)), k=1)
    return bool(np.array_equal(m, expect))


def run_on_cores(inputs, trace=False):
    """Compile+run; returns BassKernelResults."""
    from concourse.bass_utils import run_bass_kernel_spmd
    causal = _is_causal(inputs["mask"])
    use_bias = not all(
        not np.any(np.asarray(inputs[k])) for k in ("bq", "bk", "bv", "bo"))
    nc = _get_built(causal, use_bias)
    in_maps = _prep_inputs(inputs, causal, use_bias)
    r = run_bass_kernel_spmd(nc, in_maps, list(range(NCORES)), trace=trace)
    return r


def kernel(**inputs) -> np.ndarray:
    r = run_on_cores(inputs)
    out = np.empty((B, S, D), np.float32)
    for c in range(NCORES):
        b, hg = divmod(c, GPC)
        out[b, :, CW * hg:CW * (hg + 1)] = r.results[c]["out"]
    return out
